# revision 1
# baseline (speedup 1.0000x reference)
"""Gaussian splatting renderer on 8 Trainium2 NeuronCores.

Algorithm (per core, data-parallel over 16 image rows):
  - S[p, n] = -Mahalanobis^2(pixel p, gaussian n) computed as a rank-6
    matmul: S = Pfeat^T @ Gfeat where Pfeat = [x^2, xy, y^2, x, y, 1]
    per pixel and Gfeat the (negated) per-gaussian quadratic coeffs.
  - top-10 alphas per pixel == top-10 S values: DVE max8 per 256-chunk,
    merge candidates with max8 + match_replace + max8 (ranks 1..16),
    max_index against S for gaussian ids.
  - alpha = exp(0.5*S_sel) only on the selected 16 values (ACT).
  - front-to-back blend weights via a 9-step cumprod chain (batched
    across all 16 row-tiles).
  - colors fetched with gpsimd indirect_copy from a broadcast color
    table; per-partition selection recovered with a static diagonal
    mask + reduction.
"""

import os
import sys

sys.path.insert(0, "/opt/trn_rl_repo")

import numpy as np
from contextlib import ExitStack

from concourse import bacc, bass, mybir, tile
from concourse.bass_utils import run_bass_kernel_spmd

H, W, K = 128, 128, 10
N_G = 2048
N_CORES = 8
ROWS_PER_CORE = H // N_CORES          # 16
T_TILES = ROWS_PER_CORE               # one image row per tile
PPT = 128                             # pixels per tile (one row)
SLOTS = 16                            # padded top-k slots (ranks 1..16)
F32 = mybir.dt.float32
U16 = mybir.dt.uint16

NEG_SENT = -3.0e38


def build_program():
    nc = bacc.Bacc(
        "TRN2",
        target_bir_lowering=False,
        debug=False,
        num_devices=N_CORES,
    )

    means = nc.dram_tensor("g_means", [N_G, 2], F32, kind="ExternalInput")
    rots = nc.dram_tensor("g_rots", [N_G], F32, kind="ExternalInput")
    lss = nc.dram_tensor("g_ls", [N_G, 2], F32, kind="ExternalInput")
    cols = nc.dram_tensor("g_cols", [N_G, 3], F32, kind="ExternalInput")
    pfeat = nc.dram_tensor("pfeat", [T_TILES, 6, PPT], F32, kind="ExternalInput")
    dmask = nc.dram_tensor("dmask", [128, 768], F32, kind="ExternalInput")
    out = nc.dram_tensor("out", [ROWS_PER_CORE * W, 3], F32, kind="ExternalOutput")

    with tile.TileContext(nc) as tc, ExitStack() as ctx:
        persist = ctx.enter_context(tc.tile_pool(name="persist", bufs=1))
        dram_pool = ctx.enter_context(tc.tile_pool(name="dram", bufs=1, space="DRAM"))
        psum_pool = ctx.enter_context(tc.tile_pool(name="ps", bufs=2, space="PSUM"))
        work = ctx.enter_context(tc.tile_pool(name="work", bufs=3))

        # ---------------- per-gaussian prep ([128, 16] layout, n = 16p + j) ----
        mxy = persist.tile([128, 2 * 16], F32, tag="mxy")    # mx | my
        rot = persist.tile([128, 16], F32, tag="rot")
        lsxy = persist.tile([128, 2 * 16], F32, tag="lsxy")  # lsx | lsy
        m3 = means.ap().rearrange("(p j) c -> p j c", p=128)
        nc.gpsimd.dma_start(mxy[:, 0:16], m3[:, :, 0])
        nc.gpsimd.dma_start(mxy[:, 16:32], m3[:, :, 1])
        nc.gpsimd.dma_start(rot[:], rots.ap().rearrange("(p j) -> p j", p=128))
        l3 = lss.ap().rearrange("(p j) c -> p j c", p=128)
        nc.gpsimd.dma_start(lsxy[:, 0:16], l3[:, :, 0])
        nc.gpsimd.dma_start(lsxy[:, 16:32], l3[:, :, 1])

        mx, my = mxy[:, 0:16], mxy[:, 16:32]
        lsx, lsy = lsxy[:, 0:16], lsxy[:, 16:32]

        tmp = persist.tile([128, 16 * 16], F32, tag="preptmp")

        def tt(i):
            return tmp[:, 16 * i : 16 * (i + 1)]

        cosr, sinr, ivx, ivy = tt(0), tt(1), tt(2), tt(3)
        c2, s2, sc, va, vc = tt(4), tt(5), tt(6), tt(7), tt(8)
        vb, t1, t2, t3, t4 = tt(9), tt(10), tt(11), tt(12), tt(13)
        t5, t6 = tt(14), tt(15)

        Sin = mybir.ActivationFunctionType.Sin
        Exp = mybir.ActivationFunctionType.Exp
        # Scalar-engine Sin needs args in [-pi, pi]; rot is in [0, 2pi).
        # sinr := -sin(rot) = sin(rot - pi).
        # cosr := cos(rot) = sin(w), w = wrap(rot + pi/2) into (-pi, pi].
        # Downstream uses only cos^2, sin^2, sin*cos, so the sign of sinr
        # is absorbed by flipping one subtraction (vb below).
        negpi = persist.tile([128, 1], F32, tag="negpi")
        # write the constant on the scalar engine so the Sin below needs no
        # cross-engine wait (ACT instructions support only one sync wait)
        nc.scalar.activation(
            negpi[:], negpi[:], mybir.ActivationFunctionType.Copy,
            bias=float(-np.pi), scale=0.0,
        )
        nc.scalar.activation(sinr, rot[:], Sin, bias=negpi[:])
        phi = tt(10)  # reuse t1 slot before t1 is live
        nc.vector.tensor_scalar_add(phi, rot[:], float(np.pi / 2))
        msk = tt(11)
        nc.vector.tensor_scalar(
            msk, phi, float(np.pi), float(-2.0 * np.pi),
            mybir.AluOpType.is_gt, mybir.AluOpType.mult,
        )
        nc.vector.tensor_add(phi, phi, msk)
        nc.scalar.activation(cosr, phi, Sin)
        nc.scalar.activation(ivx, lsx, Exp, scale=-2.0)
        nc.scalar.activation(ivy, lsy, Exp, scale=-2.0)
        nc.vector.tensor_mul(c2, cosr, cosr)
        nc.vector.tensor_mul(s2, sinr, sinr)
        nc.vector.tensor_mul(sc, sinr, cosr)
        # a = c2*ivx + s2*ivy ; c = s2*ivx + c2*ivy ; b = sc*(ivx-ivy)
        nc.vector.tensor_mul(t1, c2, ivx)
        nc.vector.tensor_mul(t2, s2, ivy)
        nc.vector.tensor_add(va, t1, t2)
        nc.vector.tensor_mul(t1, s2, ivx)
        nc.vector.tensor_mul(t2, c2, ivy)
        nc.vector.tensor_add(vc, t1, t2)
        # vb = sin*cos*(ivx-ivy) = (sinr*cosr)*(ivy-ivx) since sinr = -sin
        nc.vector.tensor_sub(t3, ivy, ivx)
        nc.vector.tensor_mul(vb, sc, t3)

        gbuf = persist.tile([128, 6 * 16], F32, tag="gbuf")
        # G0 = -a, G1 = -2b, G2 = -c
        nc.vector.tensor_scalar_mul(gbuf[:, 0:16], va, -1.0)
        nc.vector.tensor_scalar_mul(gbuf[:, 16:32], vb, -2.0)
        nc.vector.tensor_scalar_mul(gbuf[:, 32:48], vc, -1.0)
        # G3 = 2(a mx + b my), G4 = 2(b mx + c my)
        nc.vector.tensor_mul(t1, va, mx)      # a mx
        nc.vector.tensor_mul(t2, vb, my)      # b my
        nc.vector.tensor_add(t3, t1, t2)
        nc.vector.tensor_scalar_mul(gbuf[:, 48:64], t3, 2.0)
        nc.vector.tensor_mul(t4, vb, mx)      # b mx
        nc.vector.tensor_mul(t5, vc, my)      # c my
        nc.vector.tensor_add(t3, t4, t5)
        nc.vector.tensor_scalar_mul(gbuf[:, 64:80], t3, 2.0)
        # G5 = -(a mx^2 + 2 b mx my + c my^2)
        nc.vector.tensor_mul(t6, t1, mx)      # a mx^2
        nc.vector.tensor_mul(t3, t4, my)      # b mx my
        nc.vector.tensor_mul(t4, t5, my)      # c my^2
        nc.vector.tensor_add(t6, t6, t4)
        nc.vector.tensor_add(t3, t3, t3)
        nc.vector.tensor_add(t6, t6, t3)
        nc.vector.tensor_scalar_mul(gbuf[:, 80:96], t6, -1.0)

        # round-trip through DRAM to transpose [128,16]x6 -> [6, 2048]
        gsc = dram_pool.tile([6, N_G], F32, tag="gsc")
        for f in range(6):
            nc.sync.dma_start(
                gsc[f : f + 1, :].rearrange("f (p j) -> f p j", p=128),
                gbuf[:, 16 * f : 16 * (f + 1)].unsqueeze(0).transpose([1, 0, 2]),
            )
        grhs = persist.tile([6, N_G], F32, tag="grhs")
        nc.sync.dma_start(grhs[:], gsc[:, :])

        # pixel features lhsT [6, 16*128]
        plhs = persist.tile([6, T_TILES * PPT], F32, tag="plhs")
        nc.sync.dma_start(
            plhs[:].rearrange("p (t j) -> p t j", t=T_TILES),
            pfeat.ap().rearrange("t f j -> f t j"),
        )

        # broadcast color table [128, 6144]
        coltab = persist.tile([128, N_G * 3], F32, tag="coltab")
        nc.sync.dma_start(
            coltab[:],
            cols.ap().rearrange("n c -> (n c)").unsqueeze(0).broadcast_to([128, N_G * 3]),
        )

        # static diagonal mask, replicated x16 tiles -> [128, 12288]
        dm = persist.tile([128, 768], F32, tag="dm")
        nc.sync.dma_start(dm[:], dmask.ap())
        dm16 = persist.tile([128, T_TILES * 768], F32, tag="dm16")
        for t in range(T_TILES):
            nc.vector.tensor_copy(dm16[:, 768 * t : 768 * (t + 1)], dm[:])

        # persists across the tile loop
        Vall = persist.tile([128, T_TILES * SLOTS], F32, tag="Vall")
        Iall = persist.tile([128, T_TILES * SLOTS], U16, tag="Iall")
        Aall = persist.tile([128, T_TILES * SLOTS], F32, tag="Aall")

        # ---------------- main loop over row-tiles -----------------------------
        for t in range(T_TILES):
            S = psum_pool.tile([128, N_G], F32, tag="S")
            lt = plhs[:, PPT * t : PPT * (t + 1)]
            for q in range(4):
                nc.tensor.matmul(
                    S[:, 512 * q : 512 * (q + 1)],
                    lhsT=lt,
                    rhs=grhs[:, 512 * q : 512 * (q + 1)],
                    start=True,
                    stop=True,
                )

            cand = work.tile([128, 64], F32, tag="cand")
            for q in range(8):
                nc.vector.max(cand[:, 8 * q : 8 * (q + 1)], S[:, 256 * q : 256 * (q + 1)])

            v = Vall[:, SLOTS * t : SLOTS * t + 8]
            v2 = Vall[:, SLOTS * t + 8 : SLOTS * (t + 1)]
            nc.vector.max(v, cand[:])
            candm = work.tile([128, 64], F32, tag="candm")
            nc.vector.match_replace(candm[:], v, cand[:], NEG_SENT)
            nc.vector.max(v2, candm[:])

            nc.vector.max_index(Iall[:, SLOTS * t : SLOTS * t + 8], v, S[:])
            nc.vector.max_index(Iall[:, SLOTS * t + 8 : SLOTS * (t + 1)], v2, S[:])

            nc.scalar.activation(
                Aall[:, SLOTS * t : SLOTS * (t + 1)],
                Vall[:, SLOTS * t : SLOTS * (t + 1)],
                Exp,
                scale=0.5,
            )

        # ---------------- batched epilogue -------------------------------------
        # O = 1 - alpha  (includes junk slots 10..16, harmless)
        Oall = persist.tile([128, T_TILES * SLOTS], F32, tag="Oall")
        nc.vector.tensor_scalar(
            Oall[:], Aall[:], -1.0, 1.0, mybir.AluOpType.mult, mybir.AluOpType.add
        )
        # cumprod T[k] = prod_{j<k} O[j], s-major layout [128, s(16) x t(16)]
        Tcp = persist.tile([128, SLOTS * T_TILES], F32, tag="Tcp")
        nc.vector.memset(Tcp[:, 0:16], 1.0)
        O3 = Oall[:].rearrange("p (t s) -> p s t", s=SLOTS)
        T3 = Tcp[:].rearrange("p (s t) -> p s t", t=T_TILES)
        for k in range(1, K):
            nc.vector.tensor_mul(T3[:, k : k + 1, :], T3[:, k - 1 : k, :], O3[:, k - 1 : k, :])

        Wall = persist.tile([128, T_TILES * SLOTS], F32, tag="Wall")
        nc.vector.memset(Wall[:], 0.0)
        W3 = Wall[:].rearrange("p (t s) -> p t s", s=SLOTS)
        A3 = Aall[:].rearrange("p (t s) -> p t s", s=SLOTS)
        T3b = Tcp[:].rearrange("p (s t) -> p t s", t=T_TILES)
        nc.vector.tensor_mul(W3[:, :, 0:K], A3[:, :, 0:K], T3b[:, :, 0:K])

        # color gather (indices are element ids into the [2048, 3] table)
        G4k = persist.tile([128, T_TILES * SLOTS * 16 * 3], F32, tag="G4k")
        nc.gpsimd.ap_gather(
            G4k[:].rearrange("p (i c) -> p i c", c=3),
            coltab[:].rearrange("p (n c) -> p n c", c=3),
            Iall[:].bitcast(mybir.dt.int16),
            channels=128,
            num_elems=N_G,
            d=3,
            num_idxs=T_TILES * SLOTS * 16,
        )
        # mask out non-own-partition gathers, reduce over partition-slot axis
        nc.vector.tensor_mul(G4k[:], G4k[:], dm16[:])
        D = persist.tile([128, T_TILES * SLOTS * 3], F32, tag="D")
        nc.vector.tensor_reduce(
            D[:].rearrange("p (t s c) -> p t s c", t=T_TILES, s=SLOTS),
            G4k[:].rearrange("p (t s pp c) -> p t s c pp", t=T_TILES, s=SLOTS, pp=16),
            mybir.AxisListType.X,
            mybir.AluOpType.add,
        )
        # weights broadcast over channel, multiply, reduce over slots
        W3c = persist.tile([128, T_TILES * SLOTS * 3], F32, tag="W3c")
        Wv = Wall[:].rearrange("p (t s) -> p t s", s=SLOTS).unsqueeze(3)
        W3cv = W3c[:].rearrange("p (t s c) -> p t s c", t=T_TILES, s=SLOTS)
        for c in range(3):
            nc.vector.tensor_copy(W3cv[:, :, :, c : c + 1], Wv)
        nc.vector.tensor_mul(W3c[:], W3c[:], D[:])
        outc = persist.tile([128, T_TILES * 3], F32, tag="outc")
        nc.vector.tensor_reduce(
            outc[:].rearrange("p (t c) -> p t c", t=T_TILES),
            W3c[:].rearrange("p (t s c) -> p t c s", t=T_TILES, s=SLOTS),
            mybir.AxisListType.X,
            mybir.AluOpType.add,
        )
        nc.sync.dma_start(
            out.ap().rearrange("(t j) c -> j t c", t=T_TILES),
            outc[:].rearrange("p (t c) -> p t c", t=T_TILES),
        )

    nc.compile()
    return nc


def host_constants():
    """Per-core pixel-feature matrices + the shared diagonal mask."""
    xs = np.arange(W, dtype=np.float32) + 0.5
    pfeats = []
    for core in range(N_CORES):
        pf = np.zeros((T_TILES, 6, PPT), np.float32)
        for t in range(T_TILES):
            y = np.float32(16 * core + t + 0.5)
            pf[t, 0] = xs * xs
            pf[t, 1] = xs * y
            pf[t, 2] = np.float32(y * y)
            pf[t, 3] = xs
            pf[t, 4] = y
            pf[t, 5] = 1.0
        pfeats.append(pf)
    dmask = np.zeros((128, 768), np.float32)
    p = np.arange(128)
    for s in range(SLOTS):
        for pp in range(16):
            for c in range(3):
                dmask[p % 16 == pp, 48 * s + 3 * pp + c] = 1.0
    return pfeats, dmask


_NC_CACHE = {}


def _get_nc():
    if "nc" not in _NC_CACHE:
        _NC_CACHE["nc"] = build_program()
    return _NC_CACHE["nc"]


def kernel(
    gaussian_means, gaussian_rotations, gaussian_log_scales, gaussian_colors
):
    nc = _get_nc()
    pfeats, dmask = host_constants()
    base = {
        "g_means": np.ascontiguousarray(gaussian_means, np.float32),
        "g_rots": np.ascontiguousarray(gaussian_rotations, np.float32),
        "g_ls": np.ascontiguousarray(gaussian_log_scales, np.float32),
        "g_cols": np.ascontiguousarray(gaussian_colors, np.float32),
        "dmask": dmask,
    }
    in_maps = [{**base, "pfeat": pfeats[c]} for c in range(N_CORES)]
    res = run_bass_kernel_spmd(nc, in_maps, list(range(N_CORES)))
    rows = [res.results[c]["out"].reshape(ROWS_PER_CORE, W, 3) for c in range(N_CORES)]
    return np.concatenate(rows, axis=0)


if __name__ == "__main__":
    ins = {
        "gaussian_means": np.random.rand(N_G, 2).astype(np.float32) * [W, H],
        "gaussian_rotations": np.random.rand(N_G).astype(np.float32) * 2 * np.pi,
        "gaussian_log_scales": (np.random.randn(N_G, 2) * 0.3 + np.log(3)).astype(
            np.float32
        ),
        "gaussian_colors": np.random.rand(N_G, 3).astype(np.float32),
    }
    img = kernel(**ins)
    print(img.shape, img.dtype, img.mean())



# revision 2
# speedup vs baseline: 1.5957x; 1.5957x over previous
"""Gaussian splatting renderer on 8 Trainium2 NeuronCores.

Algorithm (per core, data-parallel over 16 image rows):
  - S[p, n] = -Mahalanobis^2(pixel p, gaussian n) as a matmul over
    quadratic features. Both operands are split 3-way into bf16
    (hi/mid/lo) and 6 of the 9 cross-term blocks are kept (K=36 rows,
    still one PE pass), recovering ~fp32 accuracy at bf16 speed.
    Coordinates are centered per core (x-64, y-rowband-center) to keep
    feature magnitudes small.
  - S is copied to fp16 (scalar engine, scale 0.5 folded in) so the DVE
    runs in 2x mode: top-16 via max8 per 512-chunk, merge with
    max8 + match_replace + max8, ids via max_index.
  - alpha = exp(S_sel) batched per half on the scalar engine.
  - front-to-back blend weights via a 9-step cumprod chain; the 1/255
    color dequant factor is folded into the cumprod seed.
  - colors are packed 3xu8 into one 4-byte unit and fetched with a
    single gpsimd ap_gather per image half (d=4 u8 instead of d=3 f32
    and 10 slots instead of 16: ~5x less gather traffic); per-partition
    selection recovered with a static diagonal mask + reduction.
  - the first half's gather runs while tiles 8-15 are still computing.
"""

import os
import sys

sys.path.insert(0, "/opt/trn_rl_repo")

import numpy as np
from contextlib import ExitStack

from concourse import bacc, bass, mybir, tile
from concourse.bass_utils import run_bass_kernel_spmd

H, W, K = 128, 128, 10
N_G = 2048
N_CORES = 8
ROWS_PER_CORE = H // N_CORES          # 16
T_TILES = ROWS_PER_CORE               # one image row per tile
PPT = 128                             # pixels per tile (one row)
SLOTS = 16                            # padded top-k slots (ranks 1..16)
HT = T_TILES // 2                     # tiles per epilogue half
F32 = mybir.dt.float32
F16 = mybir.dt.float16
BF16 = mybir.dt.bfloat16
U8 = mybir.dt.uint8
U16 = mybir.dt.uint16

NEG_SENT = -60000.0


def build_program():
    nc = bacc.Bacc(
        "TRN2",
        target_bir_lowering=False,
        debug=False,
        num_devices=N_CORES,
    )

    means = nc.dram_tensor("g_means", [N_G, 2], F32, kind="ExternalInput")
    rots = nc.dram_tensor("g_rots", [N_G], F32, kind="ExternalInput")
    lss = nc.dram_tensor("g_ls", [N_G, 2], F32, kind="ExternalInput")
    colpack = nc.dram_tensor("colpack", [N_G, 4], U8, kind="ExternalInput")
    pfeat = nc.dram_tensor("pfeat", [T_TILES, 18, PPT], F32, kind="ExternalInput")
    dmask = nc.dram_tensor("dmask", [128, HT * K * 16 * 3], F32, kind="ExternalInput")
    out = nc.dram_tensor("out", [ROWS_PER_CORE * W, 3], F32, kind="ExternalOutput")

    with tile.TileContext(nc) as tc, ExitStack() as ctx:
        persist = ctx.enter_context(tc.tile_pool(name="persist", bufs=1))
        dram_pool = ctx.enter_context(tc.tile_pool(name="dram", bufs=1, space="DRAM"))
        psum_pool = ctx.enter_context(tc.tile_pool(name="ps", bufs=2, space="PSUM"))
        work = ctx.enter_context(tc.tile_pool(name="work", bufs=3))
        ework = ctx.enter_context(tc.tile_pool(name="ework", bufs=2))

        # ---------------- per-gaussian prep ([128, 16] layout, n = 16p + j) ----
        mxy = persist.tile([128, 2 * 16], F32, tag="mxy")    # mx | my
        rot = persist.tile([128, 16], F32, tag="rot")
        lsxy = persist.tile([128, 2 * 16], F32, tag="lsxy")  # lsx | lsy
        m3 = means.ap().rearrange("(p j) c -> p j c", p=128)
        nc.gpsimd.dma_start(mxy[:, 0:16], m3[:, :, 0])
        nc.gpsimd.dma_start(mxy[:, 16:32], m3[:, :, 1])
        nc.gpsimd.dma_start(rot[:], rots.ap().rearrange("(p j) -> p j", p=128))
        l3 = lss.ap().rearrange("(p j) c -> p j c", p=128)
        nc.gpsimd.dma_start(lsxy[:, 0:16], l3[:, :, 0])
        nc.gpsimd.dma_start(lsxy[:, 16:32], l3[:, :, 1])

        mx, my = mxy[:, 0:16], mxy[:, 16:32]
        lsx, lsy = lsxy[:, 0:16], lsxy[:, 16:32]

        tmp = persist.tile([128, 16 * 16], F32, tag="preptmp")

        def tt(i):
            return tmp[:, 16 * i : 16 * (i + 1)]

        cosr, sinr, ivx, ivy = tt(0), tt(1), tt(2), tt(3)
        c2, s2, sc, va, vc = tt(4), tt(5), tt(6), tt(7), tt(8)
        vb, t1, t2, t3, t4 = tt(9), tt(10), tt(11), tt(12), tt(13)
        t5, t6 = tt(14), tt(15)

        Sin = mybir.ActivationFunctionType.Sin
        Exp = mybir.ActivationFunctionType.Exp
        Copy = mybir.ActivationFunctionType.Copy
        # Scalar-engine Sin needs args in [-pi, pi]; rot is in [0, 2pi).
        # sinr := -sin(rot) = sin(rot - pi).
        # cosr := cos(rot) = sin(w), w = wrap(rot + pi/2) into (-pi, pi].
        # Downstream uses only cos^2, sin^2, sin*cos, so the sign of sinr
        # is absorbed by flipping one subtraction (vb below).
        negpi = persist.tile([128, 1], F32, tag="negpi")
        nc.scalar.activation(negpi[:], negpi[:], Copy, bias=float(-np.pi), scale=0.0)
        nc.scalar.activation(sinr, rot[:], Sin, bias=negpi[:])
        phi = tt(10)  # reuse t1 slot before t1 is live
        nc.vector.tensor_scalar_add(phi, rot[:], float(np.pi / 2))
        msk = tt(11)
        nc.vector.tensor_scalar(
            msk, phi, float(np.pi), float(-2.0 * np.pi),
            mybir.AluOpType.is_gt, mybir.AluOpType.mult,
        )
        nc.vector.tensor_add(phi, phi, msk)
        nc.scalar.activation(cosr, phi, Sin)
        nc.scalar.activation(ivx, lsx, Exp, scale=-2.0)
        nc.scalar.activation(ivy, lsy, Exp, scale=-2.0)
        nc.vector.tensor_mul(c2, cosr, cosr)
        nc.vector.tensor_mul(s2, sinr, sinr)
        nc.vector.tensor_mul(sc, sinr, cosr)
        # a = c2*ivx + s2*ivy ; c = s2*ivx + c2*ivy ; b = sc*(ivx-ivy)
        nc.vector.tensor_mul(t1, c2, ivx)
        nc.vector.tensor_mul(t2, s2, ivy)
        nc.vector.tensor_add(va, t1, t2)
        nc.vector.tensor_mul(t1, s2, ivx)
        nc.vector.tensor_mul(t2, c2, ivy)
        nc.vector.tensor_add(vc, t1, t2)
        # vb = sin*cos*(ivx-ivy) = (sinr*cosr)*(ivy-ivx) since sinr = -sin
        nc.vector.tensor_sub(t3, ivy, ivx)
        nc.vector.tensor_mul(vb, sc, t3)

        gbuf = persist.tile([128, 6 * 16], F32, tag="gbuf")
        # G0 = -a, G1 = -2b, G2 = -c
        nc.vector.tensor_scalar_mul(gbuf[:, 0:16], va, -1.0)
        nc.vector.tensor_scalar_mul(gbuf[:, 16:32], vb, -2.0)
        nc.vector.tensor_scalar_mul(gbuf[:, 32:48], vc, -1.0)
        # G3 = 2(a mx + b my), G4 = 2(b mx + c my)
        nc.vector.tensor_mul(t1, va, mx)      # a mx
        nc.vector.tensor_mul(t2, vb, my)      # b my
        nc.vector.tensor_add(t3, t1, t2)
        nc.vector.tensor_scalar_mul(gbuf[:, 48:64], t3, 2.0)
        nc.vector.tensor_mul(t4, vb, mx)      # b mx
        nc.vector.tensor_mul(t5, vc, my)      # c my
        nc.vector.tensor_add(t3, t4, t5)
        nc.vector.tensor_scalar_mul(gbuf[:, 64:80], t3, 2.0)
        # G5 = -(a mx^2 + 2 b mx my + c my^2)
        nc.vector.tensor_mul(t6, t1, mx)      # a mx^2
        nc.vector.tensor_mul(t3, t4, my)      # b mx my
        nc.vector.tensor_mul(t4, t5, my)      # c my^2
        nc.vector.tensor_add(t6, t6, t4)
        nc.vector.tensor_add(t3, t3, t3)
        nc.vector.tensor_add(t6, t6, t3)
        nc.vector.tensor_scalar_mul(gbuf[:, 80:96], t6, -1.0)

        # 3-way bf16 split: gbuf = gh + gm + gl (+ eps)
        gh = persist.tile([128, 96], BF16, tag="gh")
        gm = persist.tile([128, 96], BF16, tag="gm")
        gl = persist.tile([128, 96], BF16, tag="gl")
        g32 = persist.tile([128, 96], F32, tag="g32")
        r1 = persist.tile([128, 96], F32, tag="r1")
        nc.vector.tensor_copy(gh[:], gbuf[:])
        nc.vector.tensor_copy(g32[:], gh[:])
        nc.vector.tensor_sub(r1[:], gbuf[:], g32[:])
        nc.vector.tensor_copy(gm[:], r1[:])
        nc.vector.tensor_copy(g32[:], gm[:])
        nc.vector.tensor_sub(r1[:], r1[:], g32[:])
        nc.vector.tensor_copy(gl[:], r1[:])

        # round-trip through DRAM to transpose [128,16]x6x3 -> [18, 2048]
        gsc = dram_pool.tile([18, N_G], BF16, tag="gsc")
        for pi, part in enumerate((gh, gm, gl)):
            for f in range(6):
                nc.sync.dma_start(
                    gsc[6 * pi + f : 6 * pi + f + 1, :].rearrange(
                        "f (p j) -> f p j", p=128
                    ),
                    part[:, 16 * f : 16 * (f + 1)].unsqueeze(0).transpose([1, 0, 2]),
                )
        # rhs rows: [Gh; Gh; Gh; Gm; Gm; Gl] vs lhsT rows [Ph; Pm; Pl; Ph; Pm; Ph]
        grhs = persist.tile([36, N_G], BF16, tag="grhs")
        for dst, src in ((0, 0), (6, 0), (12, 0), (18, 6), (24, 6), (30, 12)):
            nc.sync.dma_start(grhs[dst : dst + 6, :], gsc[src : src + 6, :])

        # pixel features lhsT [36, 16*128]: rows [Ph(6); Pm(6); Pl(6); Ph; Pm; Ph]
        pstage = persist.tile([36, T_TILES * PPT], F32, tag="pstage")
        pf = pfeat.ap()
        nc.sync.dma_start(
            pstage[0:18, :].rearrange("p (t j) -> p t j", t=T_TILES),
            pf.rearrange("t f j -> f t j"),
        )
        for dst, src in ((18, 0), (24, 6), (30, 0)):
            nc.sync.dma_start(
                pstage[dst : dst + 6, :].rearrange("p (t j) -> p t j", t=T_TILES),
                pf[:, src : src + 6, :].rearrange("t f j -> f t j"),
            )
        plhs = persist.tile([36, T_TILES * PPT], BF16, tag="plhs")
        nc.vector.tensor_copy(plhs[:], pstage[:])

        # broadcast packed color table [128, 2048*4] u8
        coltab = persist.tile([128, N_G * 4], U8, tag="coltab")
        nc.sync.dma_start(
            coltab[:],
            colpack.ap().rearrange("n c -> (n c)").unsqueeze(0).broadcast_to(
                [128, N_G * 4]
            ),
        )

        # static diagonal mask for one epilogue half [128, 80*16*3]
        dm = persist.tile([128, HT * K * 16 * 3], F32, tag="dm")
        nc.sync.dma_start(dm[:], dmask.ap())

        # persists across the tile loop
        Vall = persist.tile([128, T_TILES * SLOTS], F16, tag="Vall")
        Iall = persist.tile([128, T_TILES * SLOTS], U16, tag="Iall")
        Itight = persist.tile([128, 2 * HT * K], U16, tag="Itight")
        G1 = persist.tile([128, 2 * HT * K * 16 * 4], U8, tag="G1")
        outc = persist.tile([128, T_TILES * 3], F32, tag="outc")

        def emit_gather(h):
            """Index-compact + color gather for half h (tiles 8h..8h+7)."""
            it = Itight[:, h * HT * K : (h + 1) * HT * K]
            nc.vector.tensor_copy(
                it.rearrange("p (t s) -> p t s", s=K),
                Iall[:, 128 * h : 128 * (h + 1)].rearrange(
                    "p (t s) -> p t s", s=SLOTS
                )[:, :, 0:K],
            )
            g1 = G1[:, h * HT * K * 64 : (h + 1) * HT * K * 64]
            nc.gpsimd.ap_gather(
                g1.rearrange("p (i c) -> p i c", c=4),
                coltab[:].rearrange("p (n c) -> p n c", c=4),
                it.bitcast(mybir.dt.int16),
                channels=128,
                num_elems=N_G,
                d=4,
                num_idxs=HT * K * 16,
            )

        # ---------------- main loop over row-tiles -----------------------------
        for t in range(T_TILES):
            S = psum_pool.tile([128, N_G], F32, tag="S")
            lt = plhs[:, PPT * t : PPT * (t + 1)]
            for q in range(4):
                nc.tensor.matmul(
                    S[:, 512 * q : 512 * (q + 1)],
                    lhsT=lt,
                    rhs=grhs[:, 512 * q : 512 * (q + 1)],
                    start=True,
                    stop=True,
                )
            # fp16 copy (x0.5 folded) so the DVE runs at 2x
            S16 = work.tile([128, N_G], F16, tag="S16")
            nc.scalar.activation(S16[:], S[:], Copy, scale=0.5)

            cand = work.tile([128, 32], F16, tag="cand")
            for q in range(4):
                nc.vector.max(cand[:, 8 * q : 8 * (q + 1)], S16[:, 512 * q : 512 * (q + 1)])

            v = Vall[:, SLOTS * t : SLOTS * t + 8]
            v2 = Vall[:, SLOTS * t + 8 : SLOTS * (t + 1)]
            nc.vector.max(v, cand[:])
            candm = work.tile([128, 32], F16, tag="candm")
            nc.vector.match_replace(candm[:], v, cand[:], NEG_SENT)
            nc.vector.max(v2, candm[:])

            nc.vector.max_index(Iall[:, SLOTS * t : SLOTS * t + 8], v, S16[:])
            nc.vector.max_index(Iall[:, SLOTS * t + 8 : SLOTS * (t + 1)], v2, S16[:])

            if t == HT - 1:
                emit_gather(0)

        emit_gather(1)

        # alphas, batched per half on the scalar engine (one Exp table load)
        Aall = persist.tile([128, T_TILES * SLOTS], F32, tag="Aall")
        nc.scalar.activation(Aall[:, 0:128], Vall[:, 0:128], Exp)
        nc.scalar.activation(Aall[:, 128:256], Vall[:, 128:256], Exp)

        # ---------------- per-half epilogue ------------------------------------
        for h in range(2):
            Ah = Aall[:, 128 * h : 128 * (h + 1)]
            # O = 1 - alpha
            Oh = ework.tile([128, HT * SLOTS], F32, tag="Oh")
            nc.vector.tensor_scalar(
                Oh[:], Ah, -1.0, 1.0, mybir.AluOpType.mult, mybir.AluOpType.add
            )
            # cumprod T[s] = (1/255) * prod_{j<s} O[j], s-major [128, s(16) x t(8)]
            # (the 1/255 seed dequantizes the u8 colors for free)
            Tcp = ework.tile([128, SLOTS * HT], F32, tag="Tcp")
            nc.vector.memset(Tcp[:, 0 : HT], 1.0 / 255.0)
            O3 = Oh[:].rearrange("p (t s) -> p s t", s=SLOTS)
            T3 = Tcp[:].rearrange("p (s t) -> p s t", t=HT)
            for k in range(1, K):
                nc.vector.tensor_mul(
                    T3[:, k : k + 1, :], T3[:, k - 1 : k, :], O3[:, k - 1 : k, :]
                )
            # W[t, s<K] = alpha * T
            Wh = ework.tile([128, HT * K], F32, tag="Wh")
            A3 = Ah.rearrange("p (t s) -> p t s", s=SLOTS)
            T3b = Tcp[:].rearrange("p (s t) -> p t s", t=HT)
            nc.vector.tensor_mul(
                Wh[:].rearrange("p (t s) -> p t s", s=K),
                A3[:, :, 0:K],
                T3b[:, :, 0:K],
            )

            # unpack gathered u8 colors -> f32 [p, ts(80)*16, 3]
            g1 = G1[:, h * HT * K * 64 : (h + 1) * HT * K * 64]
            U3 = ework.tile([128, HT * K * 16 * 3], F32, tag="U3")
            nc.vector.tensor_copy(
                U3[:].rearrange("p (i c) -> p i c", c=3),
                g1.rearrange("p (i c) -> p i c", c=4)[:, :, 0:3],
            )
            # mask out non-own-partition gathers, reduce over partition-slot axis
            nc.vector.tensor_mul(U3[:], U3[:], dm[:])
            D = ework.tile([128, HT * K * 3], F32, tag="D")
            nc.vector.tensor_reduce(
                D[:].rearrange("p (ts c) -> p ts c", c=3),
                U3[:].rearrange("p (ts pp c) -> p ts c pp", pp=16, c=3),
                mybir.AxisListType.X,
                mybir.AluOpType.add,
            )
            # weights broadcast over channel, multiply, reduce over slots
            W3c = ework.tile([128, HT * K * 3], F32, tag="W3c")
            Wv = Wh[:].rearrange("p (t s) -> p t s", s=K).unsqueeze(3)
            W3cv = W3c[:].rearrange("p (t s c) -> p t s c", t=HT, s=K)
            for c in range(3):
                nc.vector.tensor_copy(W3cv[:, :, :, c : c + 1], Wv)
            nc.vector.tensor_mul(W3c[:], W3c[:], D[:])
            nc.vector.tensor_reduce(
                outc[:, 24 * h : 24 * (h + 1)].rearrange("p (t c) -> p t c", c=3),
                W3c[:].rearrange("p (t s c) -> p t c s", t=HT, s=K),
                mybir.AxisListType.X,
                mybir.AluOpType.add,
            )

        nc.sync.dma_start(
            out.ap().rearrange("(t j) c -> j t c", t=T_TILES),
            outc[:].rearrange("p (t c) -> p t c", t=T_TILES),
        )

    nc.compile()
    return nc


def host_inputs():
    """Per-core pixel-feature matrices (3-way bf16 split) + the shared mask."""
    import ml_dtypes

    bf = ml_dtypes.bfloat16

    def split3(x):
        h = x.astype(bf).astype(np.float64)
        m = (x - h).astype(bf).astype(np.float64)
        l = (x - h - m).astype(bf).astype(np.float64)
        return h, m, l

    xs = np.arange(W, dtype=np.float64) + 0.5 - 64.0
    pfeats = []
    for core in range(N_CORES):
        pf = np.zeros((T_TILES, 18, PPT), np.float64)
        for t in range(T_TILES):
            y = t - 7.5  # (16c + t + 0.5) - (16c + 8)
            P = np.stack(
                [xs * xs, xs * y, np.full(PPT, y * y), xs, np.full(PPT, y), np.ones(PPT)]
            )
            Ph, Pm, Pl = split3(P)
            pf[t, 0:6] = Ph
            pf[t, 6:12] = Pm
            pf[t, 12:18] = Pl
        pfeats.append(pf.astype(np.float32))
    dmask = np.zeros((128, HT * K * 16 * 3), np.float32)
    p = np.arange(128)
    for ts in range(HT * K):
        for pp in range(16):
            dmask[p % 16 == pp, ts * 48 + pp * 3 : ts * 48 + pp * 3 + 3] = 1.0
    return pfeats, dmask


def build_in_maps(gaussian_means, gaussian_rotations, gaussian_log_scales, gaussian_colors):
    pfeats, dmask = host_inputs()
    colpack = np.zeros((N_G, 4), np.uint8)
    colpack[:, 0:3] = np.clip(
        np.round(np.asarray(gaussian_colors, np.float64) * 255.0), 0, 255
    ).astype(np.uint8)
    base = {
        "g_rots": np.ascontiguousarray(gaussian_rotations, np.float32),
        "g_ls": np.ascontiguousarray(gaussian_log_scales, np.float32),
        "colpack": colpack,
        "dmask": dmask,
    }
    in_maps = []
    for c in range(N_CORES):
        shift = np.array([64.0, 16.0 * c + 8.0], np.float32)
        in_maps.append(
            {
                **base,
                "g_means": np.ascontiguousarray(gaussian_means, np.float32) - shift,
                "pfeat": pfeats[c],
            }
        )
    return in_maps


_NC_CACHE = {}


def _get_nc():
    if "nc" not in _NC_CACHE:
        _NC_CACHE["nc"] = build_program()
    return _NC_CACHE["nc"]


def kernel(
    gaussian_means, gaussian_rotations, gaussian_log_scales, gaussian_colors
):
    nc = _get_nc()
    in_maps = build_in_maps(
        gaussian_means, gaussian_rotations, gaussian_log_scales, gaussian_colors
    )
    res = run_bass_kernel_spmd(nc, in_maps, list(range(N_CORES)))
    rows = [res.results[c]["out"].reshape(ROWS_PER_CORE, W, 3) for c in range(N_CORES)]
    return np.concatenate(rows, axis=0)


if __name__ == "__main__":
    ins = {
        "gaussian_means": np.random.rand(N_G, 2).astype(np.float32) * [W, H],
        "gaussian_rotations": np.random.rand(N_G).astype(np.float32) * 2 * np.pi,
        "gaussian_log_scales": (np.random.randn(N_G, 2) * 0.3 + np.log(3)).astype(
            np.float32
        ),
        "gaussian_colors": np.random.rand(N_G, 3).astype(np.float32),
    }
    img = kernel(**ins)
    print(img.shape, img.dtype, img.mean())
